# revision 5
# baseline (speedup 1.0000x reference)
"""GQA causal-attention prefill (B=2, T=S=2048, D=2048, N=16, K=4, H=128)
on 8 Trainium2 NeuronCores.

Sharding: one (batch, kv-head) pair per core -> 2*4 = 8 cores, zero
cross-core communication on device; the o_proj partial sums (over each
batch's 4 kv-head groups) are reduced on the host at unshard time.

Per-core dataflow (all layouts chosen so no on-device transposes of the
big operands are ever needed; host pre-transposes Xq/Xkv once):
  QT_n[h,t] = Wq_n^T X^T        (matmul lhsT=Wq slice, rhs=XqT)  + RoPE
  KT[h,s]   = Wk^T Xkv^T                                          + RoPE
  VT[h,s]   = Wv^T Xkv^T  -> V[s,h] via PE transpose
  scoresT[s,t] = KT_blk^T QT    (lhsT=KT block, rhs=QT chunk)
  probsT = exp(scoresT + causal mask)            (ACT, unnormalized)
  OT_n[h,t] += V_blk^T probsT   ;  den[1,t] += ones^T probsT
  OT_n *= broadcast(1/den)      (ones-row matmul broadcast + DVE)
  O[t,d]  = sum_n OT_n^T Wo_n   (accumulated in PSUM over heads)

Matmuls run as float32r (full-rate fp32 on TRN2 for moving dim >= 256).
"""
import numpy as np

import concourse.bass as bass
from concourse import bacc
import concourse.tile as tile
import concourse.mybir as mybir
from concourse.bass_utils import run_bass_kernel_spmd
from concourse.masks import make_identity

B, T, D = 2, 2048, 2048
N, K, H = 16, 4, 128
G = N // K
HALF = H // 2
MIN_TS, MAX_TS = 1.0, 10000.0

P = 128
TCH = 512
NCH = T // TCH          # 4 t-chunks of 512
DB = D // P             # 16 contraction blocks
F32 = mybir.dt.float32
F32R = mybir.dt.float32r
NEG = -1.0e30
EXP = mybir.ActivationFunctionType.Exp

_CACHE = {}
LAST = None             # BassKernelResults of the most recent run


def _rope_from_psum(nc, pool, ps, dst, cos_ap, sin_ap):
    """dst[128,TCH] = psum*cosF + rot(psum)*sinF, rot = swap partition halves."""
    nc.scalar.copy(dst, ps[:])
    rot = pool.tile([P, TCH], F32R, tag="rot")
    prod = pool.tile([P, TCH], F32, tag="prod")
    nc.sync.dma_start(rot[0:HALF, :], dst[HALF:P, :])
    nc.sync.dma_start(rot[HALF:P, :], dst[0:HALF, :])
    nc.vector.tensor_mul(prod[:], rot[:], sin_ap)
    nc.vector.tensor_mul(dst, dst, cos_ap)
    nc.vector.tensor_add(dst, dst, prod[:])


def _build():
    if "nc" in _CACHE:
        return _CACHE["nc"]
    nc = bacc.Bacc(None, target_bir_lowering=False, debug=False)
    xq = nc.declare_dram_parameter("xqT", [D, T], F32R, isOutput=False)
    xkv = nc.declare_dram_parameter("xkvT", [D, T], F32R, isOutput=False)
    wq = nc.declare_dram_parameter("wq", [D, G * H], F32R, isOutput=False)
    wk = nc.declare_dram_parameter("wk", [D, H], F32R, isOutput=False)
    wv = nc.declare_dram_parameter("wv", [D, H], F32R, isOutput=False)
    wo = nc.declare_dram_parameter("wo", [G, H, D], F32R, isOutput=False)
    cq = nc.declare_dram_parameter("cosq", [P, T], F32, isOutput=False)
    sq = nc.declare_dram_parameter("sinq", [P, T], F32, isOutput=False)
    ck = nc.declare_dram_parameter("cosk", [P, T], F32, isOutput=False)
    sk = nc.declare_dram_parameter("sink", [P, T], F32, isOutput=False)
    tri = nc.declare_dram_parameter("tri", [P, P], F32, isOutput=False)
    one_c = nc.declare_dram_parameter("one_c", [P, 1], F32R, isOutput=False)
    one_r = nc.declare_dram_parameter("one_r", [1, P], F32R, isOutput=False)
    out = nc.declare_dram_parameter("O", [T, D], F32, isOutput=True)

    xq_v = xq[:].rearrange("(do di) t -> di do t", di=P)
    xkv_v = xkv[:].rearrange("(do di) t -> di do t", di=P)
    wq_v = wq[:].rearrange("(do di) nh -> di do nh", di=P)
    wk_v = wk[:].rearrange("(do di) h -> di do h", di=P)
    wv_v = wv[:].rearrange("(do di) h -> di do h", di=P)
    wo_v = wo[:].rearrange("n h d -> h n d")

    with tile.TileContext(nc) as tc:
        with tc.tile_pool(name="glob", bufs=1) as glob:
            qt = glob.tile([P, G, T], F32R)
            kt = glob.tile([P, T], F32R)
            vsb = glob.tile([P, DB, H], F32R)
            tri_sb = glob.tile([P, P], F32)
            ones_col = glob.tile([P, 1], F32R)
            ones_row = glob.tile([1, P], F32R)
            ident = glob.tile([P, P], F32)
            nc.sync.dma_start(tri_sb[:], tri[:])
            nc.sync.dma_start(ones_col[:], one_c[:])
            nc.sync.dma_start(ones_row[:], one_r[:])
            make_identity(nc, ident[:])

            # ---------- Phase 1: Q projection + RoPE -> qt ----------
            with tc.tile_pool(name="p1w", bufs=1) as p1w, \
                 tc.tile_pool(name="p1x", bufs=24) as p1x, \
                 tc.tile_pool(name="p1t", bufs=3) as p1t, \
                 tc.tile_pool(name="p1ps", bufs=2, space="PSUM") as p1ps:
                wq_sb = p1w.tile([P, DB, G * H], F32R)
                cosq_sb = p1w.tile([P, T], F32)
                sinq_sb = p1w.tile([P, T], F32)
                nc.sync.dma_start(wq_sb[:], wq_v)
                nc.sync.dma_start(cosq_sb[:], cq[:])
                nc.sync.dma_start(sinq_sb[:], sq[:])
                for c in range(NCH):
                    tsl = slice(c * TCH, (c + 1) * TCH)
                    xts = []
                    for db in range(DB):
                        xt = p1x.tile([P, TCH], F32R, tag="xt")
                        nc.sync.dma_start(xt[:], xq_v[:, db, tsl])
                        xts.append(xt)
                    for n in range(G):
                        ps = p1ps.tile([P, TCH], F32, tag="qps")
                        for db in range(DB):
                            nc.tensor.matmul(
                                ps[:], wq_sb[:, db, n * H:(n + 1) * H],
                                xts[db][:],
                                start=(db == 0), stop=(db == DB - 1))
                        _rope_from_psum(nc, p1t, ps, qt[:, n, tsl],
                                        cosq_sb[:, tsl], sinq_sb[:, tsl])

            # ---------- Phase 2: K (RoPE) and V ----------
            with tc.tile_pool(name="p2w", bufs=1) as p2w, \
                 tc.tile_pool(name="p2x", bufs=24) as p2x, \
                 tc.tile_pool(name="p2t", bufs=3) as p2t, \
                 tc.tile_pool(name="p2ps", bufs=2, space="PSUM") as p2ps, \
                 tc.tile_pool(name="p2pt", bufs=2, space="PSUM") as p2pt:
                wk_sb = p2w.tile([P, DB, H], F32R)
                wv_sb = p2w.tile([P, DB, H], F32R)
                cosk_sb = p2w.tile([P, T], F32)
                sink_sb = p2w.tile([P, T], F32)
                nc.sync.dma_start(wk_sb[:], wk_v)
                nc.sync.dma_start(wv_sb[:], wv_v)
                nc.sync.dma_start(cosk_sb[:], ck[:])
                nc.sync.dma_start(sink_sb[:], sk[:])
                for c in range(NCH):
                    tsl = slice(c * TCH, (c + 1) * TCH)
                    xts = []
                    for db in range(DB):
                        xt = p2x.tile([P, TCH], F32R, tag="xt")
                        nc.sync.dma_start(xt[:], xkv_v[:, db, tsl])
                        xts.append(xt)
                    ps = p2ps.tile([P, TCH], F32, tag="kps")
                    for db in range(DB):
                        nc.tensor.matmul(ps[:], wk_sb[:, db, :],
                                         xts[db][:],
                                         start=(db == 0), stop=(db == DB - 1))
                    _rope_from_psum(nc, p2t, ps, kt[:, tsl],
                                    cosk_sb[:, tsl], sink_sb[:, tsl])
                    ps2 = p2ps.tile([P, TCH], F32, tag="vps")
                    for db in range(DB):
                        nc.tensor.matmul(ps2[:], wv_sb[:, db, :],
                                         xts[db][:],
                                         start=(db == 0), stop=(db == DB - 1))
                    vt_tmp = p2t.tile([P, TCH], F32, tag="vt")
                    nc.scalar.copy(vt_tmp[:], ps2[:])
                    for kk in range(4):
                        pst = p2pt.tile([P, P], F32, tag="tp")
                        nc.tensor.transpose(pst[:], vt_tmp[:, kk * P:(kk + 1) * P],
                                            ident[:])
                        nc.scalar.copy(vsb[:, 4 * c + kk, :], pst[:])

            # ---------- Phase 3: attention + o_proj ----------
            with tc.tile_pool(name="p3w", bufs=1) as p3w, \
                 tc.tile_pool(name="pb", bufs=4) as pbp, \
                 tc.tile_pool(name="otp", bufs=2) as otp, \
                 tc.tile_pool(name="smp", bufs=2) as smp, \
                 tc.tile_pool(name="obp", bufs=3) as obp, \
                 tc.tile_pool(name="ps_sc", bufs=2, space="PSUM") as ps_sc, \
                 tc.tile_pool(name="ps_ot", bufs=2, space="PSUM") as ps_ot, \
                 tc.tile_pool(name="ps_den", bufs=1, space="PSUM") as ps_den, \
                 tc.tile_pool(name="ps_bc", bufs=1, space="PSUM") as ps_bc, \
                 tc.tile_pool(name="ps_o", bufs=2, space="PSUM") as ps_o:
                wo_sb = p3w.tile([P, G, D], F32R)
                nc.sync.dma_start(wo_sb[:], wo_v)
                for c in range(NCH):
                    tsl = slice(c * TCH, (c + 1) * TCH)
                    J = 4 * (c + 1)
                    otc = otp.tile([P, G, TCH], F32R, tag="otc")
                    for n in range(G):
                        ot_ps = ps_ot.tile([P, TCH], F32, tag="ot")
                        den_ps = ps_den.tile([1, TCH], F32, tag="den")
                        for j in range(J):
                            d = j - 4 * c
                            lo = max(d, 0) * P
                            s_ps = ps_sc.tile([P, TCH], F32, tag="sc")
                            nc.tensor.matmul(s_ps[:, lo:], kt[:, j * P:(j + 1) * P],
                                             qt[:, n, c * TCH + lo:(c + 1) * TCH],
                                             start=True, stop=True)
                            if d >= 0:
                                nc.vector.tensor_add(
                                    s_ps[:, d * P:(d + 1) * P],
                                    s_ps[:, d * P:(d + 1) * P], tri_sb[:])
                            pb = pbp.tile([P, TCH], F32R, tag="pb")
                            nc.scalar.activation(pb[:, lo:], s_ps[:, lo:], EXP)
                            nc.tensor.matmul(ot_ps[:, lo:], vsb[:, j, :], pb[:, lo:],
                                             start=(j == 0), stop=(j == J - 1))
                            nc.tensor.matmul(den_ps[:, lo:], ones_col[:], pb[:, lo:],
                                             start=(j == 0), stop=(j == J - 1))
                        den_sb = smp.tile([1, TCH], F32, tag="den_sb")
                        nc.scalar.copy(den_sb[:], den_ps[:])
                        inv_f32 = smp.tile([1, TCH], F32, tag="inv_f32")
                        nc.vector.reciprocal(inv_f32[:], den_sb[:])
                        inv_sb = smp.tile([1, TCH], F32R, tag="inv_sb")
                        nc.scalar.copy(inv_sb[:], inv_f32[:])
                        bc_ps = ps_bc.tile([P, TCH], F32, tag="bc")
                        nc.tensor.matmul(bc_ps[:], ones_row[:], inv_sb[:],
                                         start=True, stop=True)
                        invb = pbp.tile([P, TCH], F32, tag="invb")
                        nc.scalar.copy(invb[:], bc_ps[:])
                        nc.vector.tensor_mul(otc[:, n, :], ot_ps[:], invb[:])
                    for kk in range(4):
                        row = c * TCH + kk * P
                        for dc in range(4):
                            ops = ps_o.tile([P, TCH], F32, tag="o")
                            for n in range(G):
                                nc.tensor.matmul(
                                    ops[:],
                                    otc[:, n, kk * P:(kk + 1) * P],
                                    wo_sb[:, n, dc * TCH:(dc + 1) * TCH],
                                    start=(n == 0), stop=(n == G - 1))
                            osb = obp.tile([P, TCH], F32, tag="osb")
                            nc.scalar.copy(osb[:], ops[:])
                            nc.sync.dma_start(
                                out[row:row + P, dc * TCH:(dc + 1) * TCH], osb[:])

    nc.compile()
    _CACHE["nc"] = nc
    return nc


def _rope_tables(pos):
    ts = MIN_TS * (MAX_TS / MIN_TS) ** (2.0 * np.arange(HALF) / H)
    ang = pos.astype(np.float64)[None, :] / ts[:, None]
    c, s = np.cos(ang), np.sin(ang)
    cosF = np.ascontiguousarray(np.concatenate([c, c], 0).astype(np.float32))
    sinF = np.ascontiguousarray(np.concatenate([-s, s], 0).astype(np.float32))
    return cosF, sinF


def kernel(Xq, Xkv, q_positions, kv_positions, Wq, Wk, Wv, Wo, _trace=False):
    global LAST
    nc = _build()
    Xq = np.asarray(Xq, dtype=np.float32)
    Xkv = np.asarray(Xkv, dtype=np.float32)
    Wq = np.asarray(Wq, dtype=np.float32)
    Wk = np.asarray(Wk, dtype=np.float32)
    Wv = np.asarray(Wv, dtype=np.float32)
    Wo = np.asarray(Wo, dtype=np.float32)
    qp = np.asarray(q_positions)
    kp = np.asarray(kv_positions)

    idx = np.arange(P)
    tri_np = np.where(idx[:, None] <= idx[None, :], 0.0, NEG).astype(np.float32)

    in_maps = []
    for core in range(8):
        b, kv = divmod(core, 4)
        cq_, sq_ = _rope_tables(qp[b])
        ck_, sk_ = _rope_tables(kp[b])
        in_maps.append({
            "xqT": np.ascontiguousarray(Xq[b].T),
            "xkvT": np.ascontiguousarray(Xkv[b].T),
            "wq": np.ascontiguousarray(
                Wq[:, kv * G:(kv + 1) * G, :].reshape(D, G * H)),
            "wk": np.ascontiguousarray(Wk[:, kv, :]),
            "wv": np.ascontiguousarray(Wv[:, kv, :]),
            "wo": np.ascontiguousarray(Wo[kv * G:(kv + 1) * G]),
            "cosq": cq_, "sinq": sq_, "cosk": ck_, "sink": sk_,
            "tri": tri_np,
            "one_c": np.ones((P, 1), np.float32),
            "one_r": np.ones((1, P), np.float32),
        })

    LAST = run_bass_kernel_spmd(nc, in_maps, list(range(8)), trace=_trace)
    parts = [r["O"] for r in LAST.results]
    O = np.stack([parts[0] + parts[1] + parts[2] + parts[3],
                  parts[4] + parts[5] + parts[6] + parts[7]])
    return np.ascontiguousarray(O.astype(np.float32))


# revision 6
# speedup vs baseline: 1.2349x; 1.2349x over previous
"""GQA causal-attention prefill (B=2, T=S=2048, D=2048, N=16, K=4, H=128)
on 8 Trainium2 NeuronCores.

Sharding: one (batch, kv-head) pair per core -> 2*4 = 8 cores, zero
cross-core communication on device; the o_proj partial sums (over each
batch's 4 kv-head groups) are reduced on the host at unshard time.

Per-core dataflow (all layouts chosen so no on-device transposes of the
big operands are ever needed; host pre-transposes Xq/Xkv once):
  QT_n[h,t] = Wq_n^T X^T        (matmul lhsT=Wq slice, rhs=XqT)  + RoPE
  KT[h,s]   = Wk^T Xkv^T                                          + RoPE
  VT[h,s]   = Wv^T Xkv^T  -> V[s,h] via PE transpose
  scoresT[s,t] = KT_blk^T QT    (lhsT=KT block, rhs=QT chunk)
  probsT = exp(scoresT + causal mask)            (ACT, unnormalized)
  OT_n[h,t] += V_blk^T probsT   ;  den[1,t] += ones^T probsT
  OT_n *= broadcast(1/den)      (ones-row matmul broadcast + DVE)
  O[t,d]  = sum_n OT_n^T Wo_n   (accumulated in PSUM over heads)

Matmuls run as float32r (full-rate fp32 on TRN2 for moving dim >= 256).
"""
import numpy as np

import concourse.bass as bass
from concourse import bacc
import concourse.tile as tile
import concourse.mybir as mybir
from concourse.bass_utils import run_bass_kernel_spmd
from concourse.masks import make_identity

B, T, D = 2, 2048, 2048
N, K, H = 16, 4, 128
G = N // K
HALF = H // 2
MIN_TS, MAX_TS = 1.0, 10000.0

P = 128
TCH = 512
NCH = T // TCH          # 4 t-chunks of 512
DB = D // P             # 16 contraction blocks
F32 = mybir.dt.float32
F32R = mybir.dt.float32r
NEG = -1.0e30
EXP = mybir.ActivationFunctionType.Exp

_CACHE = {}
LAST = None             # BassKernelResults of the most recent run


def _rope_from_psum(nc, pool, ps, dst, cos_ap, sin_ap):
    """dst[128,TCH] = psum*cosF + rot(psum)*sinF, rot = swap partition halves."""
    nc.scalar.copy(dst, ps[:])
    rot = pool.tile([P, TCH], F32R, tag="rot")
    prod = pool.tile([P, TCH], F32, tag="prod")
    nc.sync.dma_start(rot[0:HALF, :], dst[HALF:P, :])
    nc.sync.dma_start(rot[HALF:P, :], dst[0:HALF, :])
    nc.vector.tensor_mul(prod[:], rot[:], sin_ap)
    nc.vector.tensor_mul(dst, dst, cos_ap)
    nc.vector.tensor_add(dst, dst, prod[:])


def _build():
    if "nc" in _CACHE:
        return _CACHE["nc"]
    nc = bacc.Bacc(None, target_bir_lowering=False, debug=False)
    xq = nc.declare_dram_parameter("xqT", [D, T], F32R, isOutput=False)
    xkv = nc.declare_dram_parameter("xkvT", [D, T], F32R, isOutput=False)
    wq = nc.declare_dram_parameter("wq", [D, G * H], F32R, isOutput=False)
    wk = nc.declare_dram_parameter("wk", [D, H], F32R, isOutput=False)
    wv = nc.declare_dram_parameter("wv", [D, H], F32R, isOutput=False)
    wo = nc.declare_dram_parameter("wo", [G, H, D], F32R, isOutput=False)
    cq = nc.declare_dram_parameter("cosq", [P, T], F32, isOutput=False)
    sq = nc.declare_dram_parameter("sinq", [P, T], F32, isOutput=False)
    ck = nc.declare_dram_parameter("cosk", [P, T], F32, isOutput=False)
    sk = nc.declare_dram_parameter("sink", [P, T], F32, isOutput=False)
    tri = nc.declare_dram_parameter("tri", [P, P], F32, isOutput=False)
    one_c = nc.declare_dram_parameter("one_c", [P, 1], F32R, isOutput=False)
    one_r = nc.declare_dram_parameter("one_r", [1, P], F32R, isOutput=False)
    out = nc.declare_dram_parameter("O", [T, D], F32, isOutput=True)

    xq_v = xq[:].rearrange("(do di) t -> di do t", di=P)
    xkv_v = xkv[:].rearrange("(do di) t -> di do t", di=P)
    wq_v = wq[:].rearrange("(do di) nh -> di do nh", di=P)
    wk_v = wk[:].rearrange("(do di) h -> di do h", di=P)
    wv_v = wv[:].rearrange("(do di) h -> di do h", di=P)
    wo_v = wo[:].rearrange("n h d -> h n d")

    with tile.TileContext(nc) as tc:
        with tc.tile_pool(name="glob", bufs=1) as glob:
            qt = glob.tile([P, G, T], F32R)
            kt = glob.tile([P, T], F32R)
            vsb = glob.tile([P, DB, H], F32R)
            tri_sb = glob.tile([P, P], F32)
            ones_col = glob.tile([P, 1], F32R)
            ones_row = glob.tile([1, P], F32R)
            ident = glob.tile([P, P], F32)
            nc.gpsimd.dma_start(tri_sb[:], tri[:])
            nc.gpsimd.dma_start(ones_col[:], one_c[:])
            nc.gpsimd.dma_start(ones_row[:], one_r[:])
            make_identity(nc, ident[:])

            # ---------- Phase 1: Q projection + RoPE -> qt ----------
            with tc.tile_pool(name="xp", bufs=32) as xp:
                with tc.tile_pool(name="p1w", bufs=1) as p1w, \
                     tc.tile_pool(name="p1t", bufs=3) as p1t, \
                     tc.tile_pool(name="p1ps", bufs=4, space="PSUM") as p1ps:
                    wq_sb = p1w.tile([P, DB, G * H], F32R)
                    cosq_sb = p1w.tile([P, T], F32)
                    sinq_sb = p1w.tile([P, T], F32)
                    nc.gpsimd.dma_start(cosq_sb[:], cq[:])
                    nc.gpsimd.dma_start(sinq_sb[:], sq[:])
                    for db in range(DB):
                        nc.sync.dma_start(wq_sb[:, db], wq_v[:, db])
                    for c in range(NCH):
                        tsl = slice(c * TCH, (c + 1) * TCH)
                        xts = []
                        for db in range(DB):
                            xt = xp.tile([P, TCH], F32R, tag="xt")
                            nc.sync.dma_start(xt[:], xq_v[:, db, tsl])
                            xts.append(xt)
                        for n in range(G):
                            ps = p1ps.tile([P, TCH], F32, tag="qps")
                            for db in range(DB):
                                nc.tensor.matmul(
                                    ps[:], wq_sb[:, db, n * H:(n + 1) * H],
                                    xts[db][:],
                                    start=(db == 0), stop=(db == DB - 1))
                            _rope_from_psum(nc, p1t, ps, qt[:, n, tsl],
                                            cosq_sb[:, tsl], sinq_sb[:, tsl])

                # ---------- Phase 2: K (RoPE) and V (wo preloads here) ------
                with tc.tile_pool(name="p3w", bufs=1) as p3w:
                    wo_sb = p3w.tile([P, G, D], F32R)
                    nc.sync.dma_start(wo_sb[:], wo_v)
                    with tc.tile_pool(name="p2w", bufs=1) as p2w, \
                         tc.tile_pool(name="p2t", bufs=3) as p2t, \
                         tc.tile_pool(name="p2ps", bufs=2, space="PSUM") as p2ps, \
                         tc.tile_pool(name="p2pt", bufs=2, space="PSUM") as p2pt:
                        wk_sb = p2w.tile([P, DB, H], F32R)
                        wv_sb = p2w.tile([P, DB, H], F32R)
                        cosk_sb = p2w.tile([P, T], F32)
                        sink_sb = p2w.tile([P, T], F32)
                        nc.gpsimd.dma_start(wk_sb[:], wk_v)
                        nc.gpsimd.dma_start(wv_sb[:], wv_v)
                        nc.gpsimd.dma_start(cosk_sb[:], ck[:])
                        nc.gpsimd.dma_start(sink_sb[:], sk[:])
                        for c in range(NCH):
                            tsl = slice(c * TCH, (c + 1) * TCH)
                            xts = []
                            for db in range(DB):
                                xt = xp.tile([P, TCH], F32R, tag="xt")
                                nc.sync.dma_start(xt[:], xkv_v[:, db, tsl])
                                xts.append(xt)
                            ps = p2ps.tile([P, TCH], F32, tag="kps")
                            for db in range(DB):
                                nc.tensor.matmul(ps[:], wk_sb[:, db, :],
                                                 xts[db][:],
                                                 start=(db == 0), stop=(db == DB - 1))
                            _rope_from_psum(nc, p2t, ps, kt[:, tsl],
                                            cosk_sb[:, tsl], sink_sb[:, tsl])
                            ps2 = p2ps.tile([P, TCH], F32, tag="vps")
                            for db in range(DB):
                                nc.tensor.matmul(ps2[:], wv_sb[:, db, :],
                                                 xts[db][:],
                                                 start=(db == 0), stop=(db == DB - 1))
                            vt_tmp = p2t.tile([P, TCH], F32, tag="vt")
                            nc.scalar.copy(vt_tmp[:], ps2[:])
                            for kk in range(4):
                                pst = p2pt.tile([P, P], F32, tag="tp")
                                nc.tensor.transpose(pst[:],
                                                    vt_tmp[:, kk * P:(kk + 1) * P],
                                                    ident[:])
                                nc.scalar.copy(vsb[:, 4 * c + kk, :], pst[:])

                    # ---------- Phase 3: attention + o_proj ----------
                    with tc.tile_pool(name="pb", bufs=4) as pbp, \
                         tc.tile_pool(name="otp", bufs=2) as otp, \
                         tc.tile_pool(name="smp", bufs=2) as smp, \
                         tc.tile_pool(name="obp", bufs=3) as obp, \
                         tc.tile_pool(name="ps_sc", bufs=2, space="PSUM") as ps_sc, \
                         tc.tile_pool(name="ps_ot", bufs=2, space="PSUM") as ps_ot, \
                         tc.tile_pool(name="ps_den", bufs=1, space="PSUM") as ps_den, \
                         tc.tile_pool(name="ps_bc", bufs=1, space="PSUM") as ps_bc, \
                         tc.tile_pool(name="ps_o", bufs=2, space="PSUM") as ps_o:
                        for c in range(NCH):
                            tsl = slice(c * TCH, (c + 1) * TCH)
                            J = 4 * (c + 1)
                            otc = otp.tile([P, G, TCH], F32R, tag="otc")
                            for n in range(G):
                                ot_ps = ps_ot.tile([P, TCH], F32, tag="ot")
                                den_ps = ps_den.tile([1, TCH], F32, tag="den")
                                for j in range(J):
                                    d = j - 4 * c
                                    lo = max(d, 0) * P
                                    s_ps = ps_sc.tile([P, TCH], F32, tag="sc")
                                    nc.tensor.matmul(s_ps[:, lo:],
                                                     kt[:, j * P:(j + 1) * P],
                                                     qt[:, n, c * TCH + lo:(c + 1) * TCH],
                                                     start=True, stop=True)
                                    if d >= 0:
                                        nc.vector.tensor_add(
                                            s_ps[:, d * P:(d + 1) * P],
                                            s_ps[:, d * P:(d + 1) * P], tri_sb[:])
                                    pb = pbp.tile([P, TCH], F32R, tag="pb")
                                    nc.scalar.activation(pb[:, lo:], s_ps[:, lo:], EXP)
                                    nc.tensor.matmul(ot_ps[:, lo:], vsb[:, j, :],
                                                     pb[:, lo:],
                                                     start=(j == 0), stop=(j == J - 1))
                                    nc.tensor.matmul(den_ps[:, lo:], ones_col[:],
                                                     pb[:, lo:],
                                                     start=(j == 0), stop=(j == J - 1))
                                den_sb = smp.tile([1, TCH], F32, tag="den_sb")
                                nc.scalar.copy(den_sb[:], den_ps[:])
                                inv_f32 = smp.tile([1, TCH], F32, tag="inv_f32")
                                nc.vector.reciprocal_approx_fast(out=inv_f32[:],
                                                                 in_=den_sb[:])
                                inv_sb = smp.tile([1, TCH], F32R, tag="inv_sb")
                                nc.scalar.copy(inv_sb[:], inv_f32[:])
                                bc_ps = ps_bc.tile([P, TCH], F32, tag="bc")
                                nc.tensor.matmul(bc_ps[:], ones_row[:], inv_sb[:],
                                                 start=True, stop=True)
                                invb = pbp.tile([P, TCH], F32, tag="invb")
                                nc.scalar.copy(invb[:], bc_ps[:])
                                nc.vector.tensor_mul(otc[:, n, :], ot_ps[:], invb[:])
                            for kk in range(4):
                                row = c * TCH + kk * P
                                for dc in range(4):
                                    ops = ps_o.tile([P, TCH], F32, tag="o")
                                    for n in range(G):
                                        nc.tensor.matmul(
                                            ops[:],
                                            otc[:, n, kk * P:(kk + 1) * P],
                                            wo_sb[:, n, dc * TCH:(dc + 1) * TCH],
                                            start=(n == 0), stop=(n == G - 1))
                                    osb = obp.tile([P, TCH], F32, tag="osb")
                                    nc.scalar.copy(osb[:], ops[:])
                                    nc.sync.dma_start(
                                        out[row:row + P, dc * TCH:(dc + 1) * TCH],
                                        osb[:])

    nc.compile()
    _CACHE["nc"] = nc
    return nc


def _rope_tables(pos):
    ts = MIN_TS * (MAX_TS / MIN_TS) ** (2.0 * np.arange(HALF) / H)
    ang = pos.astype(np.float64)[None, :] / ts[:, None]
    c, s = np.cos(ang), np.sin(ang)
    cosF = np.ascontiguousarray(np.concatenate([c, c], 0).astype(np.float32))
    sinF = np.ascontiguousarray(np.concatenate([-s, s], 0).astype(np.float32))
    return cosF, sinF


def kernel(Xq, Xkv, q_positions, kv_positions, Wq, Wk, Wv, Wo, _trace=False):
    global LAST
    nc = _build()
    Xq = np.asarray(Xq, dtype=np.float32)
    Xkv = np.asarray(Xkv, dtype=np.float32)
    Wq = np.asarray(Wq, dtype=np.float32)
    Wk = np.asarray(Wk, dtype=np.float32)
    Wv = np.asarray(Wv, dtype=np.float32)
    Wo = np.asarray(Wo, dtype=np.float32)
    qp = np.asarray(q_positions)
    kp = np.asarray(kv_positions)

    idx = np.arange(P)
    tri_np = np.where(idx[:, None] <= idx[None, :], 0.0, NEG).astype(np.float32)

    in_maps = []
    for core in range(8):
        b, kv = divmod(core, 4)
        cq_, sq_ = _rope_tables(qp[b])
        ck_, sk_ = _rope_tables(kp[b])
        in_maps.append({
            "xqT": np.ascontiguousarray(Xq[b].T),
            "xkvT": np.ascontiguousarray(Xkv[b].T),
            "wq": np.ascontiguousarray(
                Wq[:, kv * G:(kv + 1) * G, :].reshape(D, G * H)),
            "wk": np.ascontiguousarray(Wk[:, kv, :]),
            "wv": np.ascontiguousarray(Wv[:, kv, :]),
            "wo": np.ascontiguousarray(Wo[kv * G:(kv + 1) * G]),
            "cosq": cq_, "sinq": sq_, "cosk": ck_, "sink": sk_,
            "tri": tri_np,
            "one_c": np.ones((P, 1), np.float32),
            "one_r": np.ones((1, P), np.float32),
        })

    LAST = run_bass_kernel_spmd(nc, in_maps, list(range(8)), trace=_trace)
    parts = [r["O"] for r in LAST.results]
    O = np.stack([parts[0] + parts[1] + parts[2] + parts[3],
                  parts[4] + parts[5] + parts[6] + parts[7]])
    return np.ascontiguousarray(O.astype(np.float32))


# revision 21
# speedup vs baseline: 1.4233x; 1.1525x over previous
"""GQA causal-attention prefill (B=2, T=S=2048, D=2048, N=16, K=4, H=128)
on 8 Trainium2 NeuronCores.

Sharding: one (batch, kv-head) pair per core -> 2*4 = 8 cores, zero
cross-core communication on device; the o_proj partial sums (over each
batch's 4 kv-head groups) are reduced on the host at unshard time.

Per-core dataflow (all layouts chosen so no on-device transposes of the
big operands are ever needed; host pre-transposes Xq/Xkv once):
  QT_n[h,t] = Wq_n^T X^T        (matmul lhsT=Wq slice, rhs=XqT)  + RoPE
  KT[h,s]   = Wk^T Xkv^T                                          + RoPE
  VT[h,s]   = Wv^T Xkv^T  -> V[s,h] via PE transpose
  scoresT[s,t] = KT_blk^T QT    (lhsT=KT block, rhs=QT chunk)
  probsT = exp(scoresT + causal mask)            (ACT, unnormalized)
  OT_n[h,t] += V_blk^T probsT   ;  den[1,t] += ones^T probsT
  OT_n *= broadcast(1/den)      (ones-row matmul broadcast + DVE)
  O[t,d]  = sum_n OT_n^T Wo_n   (accumulated in PSUM over heads)

Matmuls run as float32r (full-rate fp32 on TRN2 for moving dim >= 256).
"""
import sys
import types

import numpy as np

try:  # make trace=True degrade gracefully when axon_hooks is absent
    import antenv.axon_hooks  # noqa: F401
except Exception:
    try:
        import antenv
        _m = types.ModuleType("antenv.axon_hooks")
        _h = [None]
        _m.set_axon_ntff_profile_hook = lambda h: _h.__setitem__(0, h)
        _m.get_axon_ntff_profile_hook = lambda: _h[0]
        sys.modules["antenv.axon_hooks"] = _m
        antenv.axon_hooks = _m
    except Exception:
        pass

import concourse.bass as bass
from concourse import bacc
import concourse.tile as tile
import concourse.mybir as mybir
from concourse.bass_utils import run_bass_kernel_spmd
from concourse.masks import make_identity

B, T, D = 2, 2048, 2048
N, K, H = 16, 4, 128
G = N // K
HALF = H // 2
MIN_TS, MAX_TS = 1.0, 10000.0

P = 128
TCH = 512
NCH = T // TCH          # 4 t-chunks of 512
DB = D // P             # 16 contraction blocks
F32 = mybir.dt.float32
F32R = mybir.dt.float32r
NEG = -1.0e30
EXP = mybir.ActivationFunctionType.Exp

_CACHE = {}
LAST = None             # BassKernelResults of the most recent run


def _rope_from_psum(nc, pool, pspool, ps, dst, cos_ap, sin_ap, pi_sb):
    """dst[128,TCH] = x*cosF + rot(x)*sinF; rot via PE permutation matmul."""
    nc.scalar.copy(dst, ps[:])
    rot_ps = pspool.tile([P, TCH], F32, tag="rotps")
    nc.tensor.matmul(rot_ps[:], pi_sb, dst, start=True, stop=True)
    prod = pool.tile([P, TCH], F32, tag="prod")
    nc.vector.tensor_mul(prod[:], rot_ps[:], sin_ap)
    nc.vector.tensor_mul(dst, dst, cos_ap)
    nc.vector.tensor_add(dst, dst, prod[:])


def _build():
    if "nc" in _CACHE:
        return _CACHE["nc"]
    nc = bacc.Bacc(None, target_bir_lowering=False, debug=False)
    xq = nc.declare_dram_parameter("xqT", [D, T], F32R, isOutput=False)
    xkv = nc.declare_dram_parameter("xkvT", [D, T], F32R, isOutput=False)
    wq = nc.declare_dram_parameter("wq", [D, G * H], F32R, isOutput=False)
    wk = nc.declare_dram_parameter("wk", [D, H], F32R, isOutput=False)
    wv = nc.declare_dram_parameter("wv", [D, H], F32R, isOutput=False)
    wo = nc.declare_dram_parameter("wo", [G, H, D], F32R, isOutput=False)
    cq = nc.declare_dram_parameter("cosq", [P, T], F32, isOutput=False)
    sq = nc.declare_dram_parameter("sinq", [P, T], F32, isOutput=False)
    tri = nc.declare_dram_parameter("tri", [P, P], F32, isOutput=False)
    one_c = nc.declare_dram_parameter("one_c", [P, 1], F32R, isOutput=False)
    one_r = nc.declare_dram_parameter("one_r", [1, P], F32R, isOutput=False)
    pi = nc.declare_dram_parameter("pi", [P, P], F32R, isOutput=False)
    out = nc.declare_dram_parameter("O", [T, D], F32, isOutput=True)

    xq_v = xq[:].rearrange("(do di) t -> di do t", di=P)
    xkv_v = xkv[:].rearrange("(do di) t -> di do t", di=P)
    wq_v = wq[:].rearrange("(do di) nh -> di do nh", di=P)
    wk_v = wk[:].rearrange("(do di) h -> di do h", di=P)
    wv_v = wv[:].rearrange("(do di) h -> di do h", di=P)
    wo_v = wo[:].rearrange("n h d -> h n d")

    with tile.TileContext(nc) as tc:
        with tc.tile_pool(name="glob", bufs=1) as glob:
            qt = glob.tile([P, G, T], F32R)
            kt = glob.tile([P, T], F32R)
            vsb = glob.tile([P, DB, H], F32R)
            tri_sb = glob.tile([P, P], F32)
            ones_col = glob.tile([P, 1], F32R)
            ones_row = glob.tile([1, P], F32R)
            ident = glob.tile([P, P], F32)
            pi_sb = glob.tile([P, P], F32R)
            nc.gpsimd.dma_start(pi_sb[:], pi[:])
            nc.gpsimd.dma_start(tri_sb[:], tri[:])
            nc.gpsimd.dma_start(ones_col[:], one_c[:])
            nc.gpsimd.dma_start(ones_row[:], one_r[:])
            make_identity(nc, ident[:])
            with tc.tile_pool(name="warm", bufs=1, space="PSUM") as wps:
                wtile = wps.tile([P, 16], F32, tag="warm")
                for _ in range(24):
                    nc.tensor.matmul(wtile[:], pi_sb[:], pi_sb[:, :16],
                                     start=True, stop=True)

            # ----- Phase 1+2 merged: per-chunk Q proj + RoPE, K proj + RoPE, V
            with tc.tile_pool(name="xp", bufs=30) as xp, \
                 tc.tile_pool(name="pw", bufs=1) as pw, \
                 tc.tile_pool(name="pt", bufs=3) as pt, \
                 tc.tile_pool(name="ps_proj", bufs=3, space="PSUM") as ps_proj, \
                 tc.tile_pool(name="ps_rot", bufs=2, space="PSUM") as ps_rot, \
                 tc.tile_pool(name="ps_tp", bufs=2, space="PSUM") as ps_tp:
                wq_sb = pw.tile([P, DB, G * H], F32R)
                wk_sb = pw.tile([P, DB, H], F32R)
                wv_sb = pw.tile([P, DB, H], F32R)
                cosq_sb = pw.tile([P, T], F32)
                sinq_sb = pw.tile([P, T], F32)
                for cc in range(NCH):
                    ccs = slice(cc * TCH, (cc + 1) * TCH)
                    nc.gpsimd.dma_start(cosq_sb[:, ccs], cq[:, ccs])
                    nc.gpsimd.dma_start(sinq_sb[:, ccs], sq[:, ccs])
                nc.gpsimd.dma_start(wk_sb[:], wk_v)
                nc.gpsimd.dma_start(wv_sb[:], wv_v)
                for c in range(NCH):
                    tsl = slice(c * TCH, (c + 1) * TCH)
                    # --- Q projection for this chunk
                    xts = []
                    for db in range(DB):
                        if c == 0:
                            nc.sync.dma_start(wq_sb[:, db], wq_v[:, db])
                        xt = xp.tile([P, TCH], F32R, tag="xt")
                        nc.sync.dma_start(xt[:], xq_v[:, db, tsl])
                        xts.append(xt)
                    for n in range(G):
                        ps = ps_proj.tile([P, TCH], F32, tag="proj")
                        for db in range(DB):
                            nc.tensor.matmul(
                                ps[:], wq_sb[:, db, n * H:(n + 1) * H],
                                xts[db][:],
                                start=(db == 0), stop=(db == DB - 1))
                        _rope_from_psum(nc, pt, ps_rot, ps, qt[:, n, tsl],
                                        cosq_sb[:, tsl], sinq_sb[:, tsl],
                                        pi_sb[:])
                    # --- K/V for this chunk
                    xts = []
                    for db in range(DB):
                        xt = xp.tile([P, TCH], F32R, tag="xt")
                        nc.sync.dma_start(xt[:], xkv_v[:, db, tsl])
                        xts.append(xt)
                    ps = ps_proj.tile([P, TCH], F32, tag="proj")
                    for db in range(DB):
                        nc.tensor.matmul(ps[:], wk_sb[:, db, :], xts[db][:],
                                         start=(db == 0), stop=(db == DB - 1))
                    _rope_from_psum(nc, pt, ps_rot, ps, kt[:, tsl],
                                    cosq_sb[:, tsl], sinq_sb[:, tsl], pi_sb[:])
                    ps2 = ps_proj.tile([P, TCH], F32, tag="proj")
                    for db in range(DB):
                        nc.tensor.matmul(ps2[:], wv_sb[:, db, :], xts[db][:],
                                         start=(db == 0), stop=(db == DB - 1))
                    vt_tmp = pt.tile([P, TCH], F32, tag="vt")
                    nc.scalar.copy(vt_tmp[:], ps2[:])
                    for kk in range(4):
                        pst = ps_tp.tile([P, P], F32, tag="tp")
                        nc.tensor.transpose(pst[:], vt_tmp[:, kk * P:(kk + 1) * P],
                                            ident[:])
                        nc.scalar.copy(vsb[:, 4 * c + kk, :], pst[:])

            # ---------- Phase 3: attention + o_proj ----------
            with tc.tile_pool(name="p3w", bufs=1) as p3w, \
                 tc.tile_pool(name="pb", bufs=6) as pbp, \
                 tc.tile_pool(name="otp", bufs=2) as otp, \
                 tc.tile_pool(name="smp", bufs=2) as smp, \
                 tc.tile_pool(name="obp", bufs=3) as obp, \
                 tc.tile_pool(name="ps_sc", bufs=2, space="PSUM") as ps_sc, \
                 tc.tile_pool(name="ps_ot", bufs=2, space="PSUM") as ps_ot, \
                 tc.tile_pool(name="ps_den", bufs=1, space="PSUM") as ps_den, \
                 tc.tile_pool(name="ps_bc", bufs=1, space="PSUM") as ps_bc, \
                 tc.tile_pool(name="ps_o", bufs=2, space="PSUM") as ps_o:
                wo_sb = p3w.tile([P, G, D], F32R)
                nc.sync.dma_start(wo_sb[:], wo_v)
                for c in range(NCH):
                    tsl = slice(c * TCH, (c + 1) * TCH)
                    J = 4 * (c + 1)
                    otc = otp.tile([P, G, TCH], F32R, tag="otc")
                    for n in range(G):
                        ot_ps = ps_ot.tile([P, TCH], F32, tag="ot")
                        den_ps = ps_den.tile([1, TCH], F32, tag="den")
                        for j in range(J):
                            d = j - 4 * c
                            lo = max(d, 0) * P
                            s_ps = ps_sc.tile([P, TCH], F32, tag="sc")
                            nc.tensor.matmul(s_ps[:, lo:],
                                             kt[:, j * P:(j + 1) * P],
                                             qt[:, n, c * TCH + lo:(c + 1) * TCH],
                                             start=True, stop=True)
                            if d >= 0:
                                nc.vector.tensor_add(
                                    s_ps[:, d * P:(d + 1) * P],
                                    s_ps[:, d * P:(d + 1) * P], tri_sb[:])
                            pb = pbp.tile([P, TCH], F32R, tag="pb")
                            nc.scalar.activation(pb[:, lo:], s_ps[:, lo:], EXP)
                            nc.tensor.matmul(ot_ps[:, lo:], vsb[:, j, :],
                                             pb[:, lo:],
                                             start=(j == 0), stop=(j == J - 1))
                            nc.tensor.matmul(den_ps[:, lo:], ones_col[:],
                                             pb[:, lo:],
                                             start=(j == 0), stop=(j == J - 1))
                        den_sb = smp.tile([1, TCH], F32, tag="den_sb")
                        nc.vector.tensor_copy(den_sb[:], den_ps[:])
                        inv_f32 = smp.tile([1, TCH], F32, tag="inv_f32")
                        nc.vector.reciprocal_approx_fast(out=inv_f32[:],
                                                         in_=den_sb[:])
                        inv_sb = smp.tile([1, TCH], F32R, tag="inv_sb")
                        nc.vector.tensor_copy(inv_sb[:], inv_f32[:])
                        bc_ps = ps_bc.tile([P, TCH], F32, tag="bc")
                        nc.tensor.matmul(bc_ps[:], ones_row[:], inv_sb[:],
                                         start=True, stop=True)
                        invb = pbp.tile([P, TCH], F32, tag="invb")
                        nc.vector.tensor_copy(invb[:], bc_ps[:])
                        nc.vector.tensor_mul(otc[:, n, :], ot_ps[:], invb[:])
                    for kk in range(4):
                        row = c * TCH + kk * P
                        for dc in range(4):
                            ops = ps_o.tile([P, TCH], F32, tag="o")
                            for n in range(G):
                                nc.tensor.matmul(
                                    ops[:],
                                    otc[:, n, kk * P:(kk + 1) * P],
                                    wo_sb[:, n, dc * TCH:(dc + 1) * TCH],
                                    start=(n == 0), stop=(n == G - 1))
                            osb = obp.tile([P, TCH], F32, tag="osb")
                            nc.scalar.copy(osb[:], ops[:])
                            nc.sync.dma_start(
                                out[row:row + P, dc * TCH:(dc + 1) * TCH],
                                osb[:])

    nc.compile()
    _CACHE["nc"] = nc
    return nc


def _rope_tables(pos):
    ts = MIN_TS * (MAX_TS / MIN_TS) ** (2.0 * np.arange(HALF) / H)
    ang = pos.astype(np.float64)[None, :] / ts[:, None]
    c, s = np.cos(ang), np.sin(ang)
    cosF = np.ascontiguousarray(np.concatenate([c, c], 0).astype(np.float32))
    sinF = np.ascontiguousarray(np.concatenate([-s, s], 0).astype(np.float32))
    return cosF, sinF


def kernel(Xq, Xkv, q_positions, kv_positions, Wq, Wk, Wv, Wo, _trace=False):
    global LAST
    nc = _build()
    Xq = np.asarray(Xq, dtype=np.float32)
    Xkv = np.asarray(Xkv, dtype=np.float32)
    Wq = np.asarray(Wq, dtype=np.float32)
    Wk = np.asarray(Wk, dtype=np.float32)
    Wv = np.asarray(Wv, dtype=np.float32)
    Wo = np.asarray(Wo, dtype=np.float32)
    qp = np.asarray(q_positions)
    kp = np.asarray(kv_positions)
    assert np.array_equal(qp, kp), (
        "kernel assumes q_positions == kv_positions (RoPE tables shared)")

    idx = np.arange(P)
    tri_np = np.where(idx[:, None] <= idx[None, :], 0.0, NEG).astype(np.float32)
    pi_np = np.zeros((P, P), np.float32)
    pi_np[(idx + HALF) % P, idx] = 1.0

    in_maps = []
    for core in range(8):
        b, kv = divmod(core, 4)
        cq_, sq_ = _rope_tables(qp[b])
        in_maps.append({
            "xqT": np.ascontiguousarray(Xq[b].T),
            "xkvT": np.ascontiguousarray(Xkv[b].T),
            "wq": np.ascontiguousarray(
                Wq[:, kv * G:(kv + 1) * G, :].reshape(D, G * H)),
            "wk": np.ascontiguousarray(Wk[:, kv, :]),
            "wv": np.ascontiguousarray(Wv[:, kv, :]),
            "wo": np.ascontiguousarray(Wo[kv * G:(kv + 1) * G]),
            "cosq": cq_, "sinq": sq_,
            "tri": tri_np,
            "one_c": np.ones((P, 1), np.float32),
            "one_r": np.ones((1, P), np.float32),
            "pi": pi_np,
        })

    LAST = run_bass_kernel_spmd(nc, in_maps, list(range(8)), trace=_trace)
    parts = [r["O"] for r in LAST.results]
    O = np.stack([parts[0] + parts[1] + parts[2] + parts[3],
                  parts[4] + parts[5] + parts[6] + parts[7]])
    return np.ascontiguousarray(O.astype(np.float32))


# revision 22
# speedup vs baseline: 1.5468x; 1.0868x over previous
"""GQA causal-attention prefill (B=2, T=S=2048, D=2048, N=16, K=4, H=128)
on 8 Trainium2 NeuronCores.

Sharding: one (batch, kv-head) pair per core -> 2*4 = 8 cores, zero
cross-core communication on device; the o_proj partial sums (over each
batch's 4 kv-head groups) are reduced on the host at unshard time.

Per-core dataflow (all layouts chosen so no on-device transposes of the
big operands are ever needed; host pre-transposes Xq/Xkv once):
  QT_n[h,t] = Wq_n^T X^T        (matmul lhsT=Wq slice, rhs=XqT)  + RoPE
  KT[h,s]   = Wk^T Xkv^T                                          + RoPE
  VT[h,s]   = Wv^T Xkv^T  -> V[s,h] via PE transpose
  scoresT[s,t] = KT_blk^T QT    (lhsT=KT block, rhs=QT chunk)
  probsT = exp(scoresT + causal mask)            (ACT, unnormalized)
  OT_n[h,t] += V_blk^T probsT   ;  den[1,t] += ones^T probsT
  OT_n *= broadcast(1/den)      (ones-row matmul broadcast + DVE)
  O[t,d]  = sum_n OT_n^T Wo_n   (accumulated in PSUM over heads)

Matmuls run as float32r (full-rate fp32 on TRN2 for moving dim >= 256).
"""
import sys
import types

import numpy as np

try:  # make trace=True degrade gracefully when axon_hooks is absent
    import antenv.axon_hooks  # noqa: F401
except Exception:
    try:
        import antenv
        _m = types.ModuleType("antenv.axon_hooks")
        _h = [None]
        _m.set_axon_ntff_profile_hook = lambda h: _h.__setitem__(0, h)
        _m.get_axon_ntff_profile_hook = lambda: _h[0]
        sys.modules["antenv.axon_hooks"] = _m
        antenv.axon_hooks = _m
    except Exception:
        pass

import concourse.bass as bass
from concourse import bacc
import concourse.tile as tile
import concourse.mybir as mybir
from concourse.bass_utils import run_bass_kernel_spmd
from concourse.masks import make_identity

B, T, D = 2, 2048, 2048
N, K, H = 16, 4, 128
G = N // K
HALF = H // 2
MIN_TS, MAX_TS = 1.0, 10000.0

P = 128
TCH = 512
NCH = T // TCH          # 4 t-chunks of 512
DB = D // P             # 16 contraction blocks
F32 = mybir.dt.float32
F32R = mybir.dt.float32r
NEG = -1.0e30
EXP = mybir.ActivationFunctionType.Exp

_CACHE = {}
LAST = None             # BassKernelResults of the most recent run


def _rope_from_psum(nc, pool, pspool, ps, dst, cos_ap, sin_ap, pi_sb):
    """dst[128,TCH] = x*cosF + rot(x)*sinF; rot via PE permutation matmul."""
    nc.scalar.copy(dst, ps[:])
    rot_ps = pspool.tile([P, TCH], F32, tag="rotps")
    nc.tensor.matmul(rot_ps[:], pi_sb, dst, start=True, stop=True)
    prod = pool.tile([P, TCH], F32, tag="prod")
    nc.vector.tensor_mul(prod[:], rot_ps[:], sin_ap)
    nc.vector.tensor_mul(dst, dst, cos_ap)
    nc.vector.tensor_add(dst, dst, prod[:])


def _build():
    if "nc" in _CACHE:
        return _CACHE["nc"]
    nc = bacc.Bacc(None, target_bir_lowering=False, debug=False)
    xq = nc.declare_dram_parameter("xqT", [D, T], F32R, isOutput=False)
    xkv = nc.declare_dram_parameter("xkvT", [D, T], F32R, isOutput=False)
    wq = nc.declare_dram_parameter("wq", [D, G * H], F32R, isOutput=False)
    wk = nc.declare_dram_parameter("wk", [D, H], F32R, isOutput=False)
    wv = nc.declare_dram_parameter("wv", [D, H], F32R, isOutput=False)
    wo = nc.declare_dram_parameter("wo", [G, H, D], F32R, isOutput=False)
    cq = nc.declare_dram_parameter("cosq", [P, T], F32, isOutput=False)
    sq = nc.declare_dram_parameter("sinq", [P, T], F32, isOutput=False)
    tri = nc.declare_dram_parameter("tri", [P, P], F32, isOutput=False)
    one_c = nc.declare_dram_parameter("one_c", [P, 1], F32R, isOutput=False)
    one_r = nc.declare_dram_parameter("one_r", [1, P], F32R, isOutput=False)
    pi = nc.declare_dram_parameter("pi", [P, P], F32R, isOutput=False)
    out = nc.declare_dram_parameter("O", [T, D], F32, isOutput=True)

    xq_v = xq[:].rearrange("(do di) t -> di do t", di=P)
    xkv_v = xkv[:].rearrange("(do di) t -> di do t", di=P)
    wq_v = wq[:].rearrange("(do di) nh -> di do nh", di=P)
    wk_v = wk[:].rearrange("(do di) h -> di do h", di=P)
    wv_v = wv[:].rearrange("(do di) h -> di do h", di=P)
    wo_v = wo[:].rearrange("n h d -> h n d")

    with tile.TileContext(nc) as tc:
        with tc.tile_pool(name="glob", bufs=1) as glob:
            qt = glob.tile([P, G, T], F32R)
            kt = glob.tile([P, T], F32R)
            vsb = glob.tile([P, DB, H], F32R)
            tri_sb = glob.tile([P, P], F32)
            ones_col = glob.tile([P, 1], F32R)
            ones_row = glob.tile([1, P], F32R)
            ident = glob.tile([P, P], F32)
            pi_sb = glob.tile([P, P], F32R)
            nc.gpsimd.dma_start(pi_sb[:], pi[:])
            nc.gpsimd.dma_start(tri_sb[:], tri[:])
            nc.gpsimd.dma_start(ones_col[:], one_c[:])
            nc.gpsimd.dma_start(ones_row[:], one_r[:])
            make_identity(nc, ident[:])
            with tc.tile_pool(name="warm", bufs=1, space="PSUM") as wps:
                wtile = wps.tile([P, 16], F32, tag="warm")
                for _ in range(24):
                    nc.tensor.matmul(wtile[:], pi_sb[:], pi_sb[:, :16],
                                     start=True, stop=True)

            # ----- Phase 1+2 merged: per-chunk Q proj + RoPE, K proj + RoPE, V
            with tc.tile_pool(name="xp", bufs=30) as xp, \
                 tc.tile_pool(name="pw", bufs=1) as pw, \
                 tc.tile_pool(name="pt", bufs=3) as pt, \
                 tc.tile_pool(name="ps_proj", bufs=3, space="PSUM") as ps_proj, \
                 tc.tile_pool(name="ps_rot", bufs=2, space="PSUM") as ps_rot, \
                 tc.tile_pool(name="ps_tp", bufs=2, space="PSUM") as ps_tp:
                wq_sb = pw.tile([P, DB, G * H], F32R)
                wk_sb = pw.tile([P, DB, H], F32R)
                wv_sb = pw.tile([P, DB, H], F32R)
                cosq_sb = pw.tile([P, T], F32)
                sinq_sb = pw.tile([P, T], F32)
                for cc in range(NCH):
                    ccs = slice(cc * TCH, (cc + 1) * TCH)
                    nc.gpsimd.dma_start(cosq_sb[:, ccs], cq[:, ccs])
                    nc.gpsimd.dma_start(sinq_sb[:, ccs], sq[:, ccs])
                nc.gpsimd.dma_start(wk_sb[:], wk_v)
                nc.gpsimd.dma_start(wv_sb[:], wv_v)
                for c in range(NCH):
                    tsl = slice(c * TCH, (c + 1) * TCH)
                    # --- Q projection for this chunk
                    xts = []
                    for db in range(DB):
                        if c == 0:
                            nc.sync.dma_start(wq_sb[:, db], wq_v[:, db])
                        xt = xp.tile([P, TCH], F32R, tag="xt")
                        nc.sync.dma_start(xt[:], xq_v[:, db, tsl])
                        xts.append(xt)
                    for n in range(G):
                        ps = ps_proj.tile([P, TCH], F32, tag="proj")
                        for db in range(DB):
                            nc.tensor.matmul(
                                ps[:], wq_sb[:, db, n * H:(n + 1) * H],
                                xts[db][:],
                                start=(db == 0), stop=(db == DB - 1))
                        _rope_from_psum(nc, pt, ps_rot, ps, qt[:, n, tsl],
                                        cosq_sb[:, tsl], sinq_sb[:, tsl],
                                        pi_sb[:])
                    # --- K/V for this chunk
                    xts = []
                    for db in range(DB):
                        xt = xp.tile([P, TCH], F32R, tag="xt")
                        nc.sync.dma_start(xt[:], xkv_v[:, db, tsl])
                        xts.append(xt)
                    ps = ps_proj.tile([P, TCH], F32, tag="proj")
                    for db in range(DB):
                        nc.tensor.matmul(ps[:], wk_sb[:, db, :], xts[db][:],
                                         start=(db == 0), stop=(db == DB - 1))
                    _rope_from_psum(nc, pt, ps_rot, ps, kt[:, tsl],
                                    cosq_sb[:, tsl], sinq_sb[:, tsl], pi_sb[:])
                    ps2 = ps_proj.tile([P, TCH], F32, tag="proj")
                    for db in range(DB):
                        nc.tensor.matmul(ps2[:], wv_sb[:, db, :], xts[db][:],
                                         start=(db == 0), stop=(db == DB - 1))
                    vt_tmp = pt.tile([P, TCH], F32, tag="vt")
                    nc.scalar.copy(vt_tmp[:], ps2[:])
                    for kk in range(4):
                        pst = ps_tp.tile([P, P], F32, tag="tp")
                        nc.tensor.transpose(pst[:], vt_tmp[:, kk * P:(kk + 1) * P],
                                            ident[:])
                        nc.scalar.copy(vsb[:, 4 * c + kk, :], pst[:])

            # ---------- Phase 3: attention + o_proj ----------
            with tc.tile_pool(name="p3w", bufs=1) as p3w, \
                 tc.tile_pool(name="pb", bufs=6) as pbp, \
                 tc.tile_pool(name="otp", bufs=2) as otp, \
                 tc.tile_pool(name="smp", bufs=2) as smp, \
                 tc.tile_pool(name="obp", bufs=3) as obp, \
                 tc.tile_pool(name="ps_sc", bufs=3, space="PSUM") as ps_sc, \
                 tc.tile_pool(name="ps_ot", bufs=2, space="PSUM") as ps_ot, \
                 tc.tile_pool(name="ps_den", bufs=1, space="PSUM") as ps_den, \
                 tc.tile_pool(name="ps_o", bufs=2, space="PSUM") as ps_o:
                wo_sb = p3w.tile([P, G, D], F32R)
                nc.sync.dma_start(wo_sb[:], wo_v)
                for c in range(NCH):
                    tsl = slice(c * TCH, (c + 1) * TCH)
                    J = 4 * (c + 1)
                    otc = otp.tile([P, G, TCH], F32R, tag="otc")
                    for n in range(G):
                        ot_ps = ps_ot.tile([P, TCH], F32, tag="ot")
                        den_ps = ps_den.tile([1, TCH], F32, tag="den")
                        for j in range(J):
                            d = j - 4 * c
                            lo = max(d, 0) * P
                            s_ps = ps_sc.tile([P, TCH], F32, tag="sc")
                            nc.tensor.matmul(s_ps[:, lo:],
                                             kt[:, j * P:(j + 1) * P],
                                             qt[:, n, c * TCH + lo:(c + 1) * TCH],
                                             start=True, stop=True)
                            if d >= 0:
                                nc.vector.tensor_add(
                                    s_ps[:, d * P:(d + 1) * P],
                                    s_ps[:, d * P:(d + 1) * P], tri_sb[:])
                            pb = pbp.tile([P, TCH], F32R, tag="pb")
                            nc.scalar.activation(pb[:, lo:], s_ps[:, lo:], EXP)
                            nc.tensor.matmul(ot_ps[:, lo:], vsb[:, j, :],
                                             pb[:, lo:],
                                             start=(j == 0), stop=(j == J - 1))
                            nc.tensor.matmul(den_ps[:, lo:], ones_col[:],
                                             pb[:, lo:],
                                             start=(j == 0), stop=(j == J - 1))
                        den_sb = smp.tile([1, TCH], F32, tag="den_sb")
                        nc.vector.tensor_copy(den_sb[:], den_ps[:])
                        inv_f32 = smp.tile([1, TCH], F32, tag="inv_f32")
                        nc.vector.reciprocal_approx_fast(out=inv_f32[:],
                                                         in_=den_sb[:])
                        inv_sb = smp.tile([1, TCH], F32R, tag="inv_sb")
                        nc.vector.tensor_copy(inv_sb[:], inv_f32[:])
                        bc_ps = ps_o.tile([P, TCH], F32, tag="o")
                        nc.tensor.matmul(bc_ps[:], ones_row[:], inv_sb[:],
                                         start=True, stop=True)
                        invb = pbp.tile([P, TCH], F32, tag="invb")
                        nc.vector.tensor_copy(invb[:], bc_ps[:])
                        nc.vector.tensor_mul(otc[:, n, :], ot_ps[:], invb[:])
                    for kk in range(4):
                        row = c * TCH + kk * P
                        for dc in range(4):
                            ops = ps_o.tile([P, TCH], F32, tag="o")
                            for n in range(G):
                                nc.tensor.matmul(
                                    ops[:],
                                    otc[:, n, kk * P:(kk + 1) * P],
                                    wo_sb[:, n, dc * TCH:(dc + 1) * TCH],
                                    start=(n == 0), stop=(n == G - 1))
                            osb = obp.tile([P, TCH], F32, tag="osb")
                            nc.scalar.copy(osb[:], ops[:])
                            nc.sync.dma_start(
                                out[row:row + P, dc * TCH:(dc + 1) * TCH],
                                osb[:])

    nc.compile()
    _CACHE["nc"] = nc
    return nc


def _rope_tables(pos):
    ts = MIN_TS * (MAX_TS / MIN_TS) ** (2.0 * np.arange(HALF) / H)
    ang = pos.astype(np.float64)[None, :] / ts[:, None]
    c, s = np.cos(ang), np.sin(ang)
    cosF = np.ascontiguousarray(np.concatenate([c, c], 0).astype(np.float32))
    sinF = np.ascontiguousarray(np.concatenate([-s, s], 0).astype(np.float32))
    return cosF, sinF


def kernel(Xq, Xkv, q_positions, kv_positions, Wq, Wk, Wv, Wo, _trace=False):
    global LAST
    nc = _build()
    Xq = np.asarray(Xq, dtype=np.float32)
    Xkv = np.asarray(Xkv, dtype=np.float32)
    Wq = np.asarray(Wq, dtype=np.float32)
    Wk = np.asarray(Wk, dtype=np.float32)
    Wv = np.asarray(Wv, dtype=np.float32)
    Wo = np.asarray(Wo, dtype=np.float32)
    qp = np.asarray(q_positions)
    kp = np.asarray(kv_positions)
    assert np.array_equal(qp, kp), (
        "kernel assumes q_positions == kv_positions (RoPE tables shared)")

    idx = np.arange(P)
    tri_np = np.where(idx[:, None] <= idx[None, :], 0.0, NEG).astype(np.float32)
    pi_np = np.zeros((P, P), np.float32)
    pi_np[(idx + HALF) % P, idx] = 1.0

    in_maps = []
    for core in range(8):
        b, kv = divmod(core, 4)
        cq_, sq_ = _rope_tables(qp[b])
        in_maps.append({
            "xqT": np.ascontiguousarray(Xq[b].T),
            "xkvT": np.ascontiguousarray(Xkv[b].T),
            "wq": np.ascontiguousarray(
                Wq[:, kv * G:(kv + 1) * G, :].reshape(D, G * H)),
            "wk": np.ascontiguousarray(Wk[:, kv, :]),
            "wv": np.ascontiguousarray(Wv[:, kv, :]),
            "wo": np.ascontiguousarray(Wo[kv * G:(kv + 1) * G]),
            "cosq": cq_, "sinq": sq_,
            "tri": tri_np,
            "one_c": np.ones((P, 1), np.float32),
            "one_r": np.ones((1, P), np.float32),
            "pi": pi_np,
        })

    LAST = run_bass_kernel_spmd(nc, in_maps, list(range(8)), trace=_trace)
    parts = [r["O"] for r in LAST.results]
    O = np.stack([parts[0] + parts[1] + parts[2] + parts[3],
                  parts[4] + parts[5] + parts[6] + parts[7]])
    return np.ascontiguousarray(O.astype(np.float32))


# revision 23
# speedup vs baseline: 1.5875x; 1.0263x over previous
"""GQA causal-attention prefill (B=2, T=S=2048, D=2048, N=16, K=4, H=128)
on 8 Trainium2 NeuronCores.

Sharding: one (batch, kv-head) pair per core -> 2*4 = 8 cores, zero
cross-core communication on device; the o_proj partial sums (over each
batch's 4 kv-head groups) are reduced on the host at unshard time.

Per-core dataflow (all layouts chosen so no on-device transposes of the
big operands are ever needed; host pre-transposes Xq/Xkv once):
  QT_n[h,t] = Wq_n^T X^T        (matmul lhsT=Wq slice, rhs=XqT)  + RoPE
  KT[h,s]   = Wk^T Xkv^T                                          + RoPE
  VT[h,s]   = Wv^T Xkv^T  -> V[s,h] via PE transpose
  scoresT[s,t] = KT_blk^T QT    (lhsT=KT block, rhs=QT chunk)
  probsT = exp(scoresT + causal mask)            (ACT, unnormalized)
  OT_n[h,t] += V_blk^T probsT   ;  den[1,t] += ones^T probsT
  OT_n *= broadcast(1/den)      (ones-row matmul broadcast + DVE)
  O[t,d]  = sum_n OT_n^T Wo_n   (accumulated in PSUM over heads)

Matmuls run as float32r (full-rate fp32 on TRN2 for moving dim >= 256).
"""
import sys
import types

import numpy as np

try:  # make trace=True degrade gracefully when axon_hooks is absent
    import antenv.axon_hooks  # noqa: F401
except Exception:
    try:
        import antenv
        _m = types.ModuleType("antenv.axon_hooks")
        _h = [None]
        _m.set_axon_ntff_profile_hook = lambda h: _h.__setitem__(0, h)
        _m.get_axon_ntff_profile_hook = lambda: _h[0]
        sys.modules["antenv.axon_hooks"] = _m
        antenv.axon_hooks = _m
    except Exception:
        pass

import concourse.bass as bass
from concourse import bacc
import concourse.tile as tile
import concourse.mybir as mybir
from concourse.bass_utils import run_bass_kernel_spmd
from concourse.masks import make_identity

B, T, D = 2, 2048, 2048
N, K, H = 16, 4, 128
G = N // K
HALF = H // 2
MIN_TS, MAX_TS = 1.0, 10000.0

P = 128
TCH = 512
NCH = T // TCH          # 4 t-chunks of 512
DB = D // P             # 16 contraction blocks
F32 = mybir.dt.float32
F32R = mybir.dt.float32r
NEG = -1.0e30
EXP = mybir.ActivationFunctionType.Exp

_CACHE = {}
LAST = None             # BassKernelResults of the most recent run


def _rope_from_psum(nc, pool, pspool, ps, dst, cos_ap, sin_ap, pi_sb):
    """dst[128,TCH] = x*cosF + rot(x)*sinF; rot via PE permutation matmul."""
    nc.scalar.copy(dst, ps[:])
    rot_ps = pspool.tile([P, TCH], F32, tag="rotps")
    nc.tensor.matmul(rot_ps[:], pi_sb, dst, start=True, stop=True)
    prod = pool.tile([P, TCH], F32, tag="prod")
    nc.vector.tensor_mul(prod[:], rot_ps[:], sin_ap)
    nc.vector.tensor_mul(dst, dst, cos_ap)
    nc.vector.tensor_add(dst, dst, prod[:])


def _build():
    if "nc" in _CACHE:
        return _CACHE["nc"]
    nc = bacc.Bacc(None, target_bir_lowering=False, debug=False)
    xq = nc.declare_dram_parameter("xqT", [D, T], F32R, isOutput=False)
    xkv = nc.declare_dram_parameter("xkvT", [D, T], F32R, isOutput=False)
    wq = nc.declare_dram_parameter("wq", [D, G * H], F32R, isOutput=False)
    wk = nc.declare_dram_parameter("wk", [D, H], F32R, isOutput=False)
    wv = nc.declare_dram_parameter("wv", [D, H], F32R, isOutput=False)
    wo = nc.declare_dram_parameter("wo", [G, H, D], F32R, isOutput=False)
    cq = nc.declare_dram_parameter("cosq", [P, T], F32, isOutput=False)
    sq = nc.declare_dram_parameter("sinq", [P, T], F32, isOutput=False)
    tri = nc.declare_dram_parameter("tri", [P, P], F32, isOutput=False)
    one_c = nc.declare_dram_parameter("one_c", [P, 1], F32R, isOutput=False)
    one_r = nc.declare_dram_parameter("one_r", [1, P], F32R, isOutput=False)
    pi = nc.declare_dram_parameter("pi", [P, P], F32R, isOutput=False)
    out = nc.declare_dram_parameter("O", [T, D], F32, isOutput=True)

    xq_v = xq[:].rearrange("(do di) t -> di do t", di=P)
    xkv_v = xkv[:].rearrange("(do di) t -> di do t", di=P)
    wq_v = wq[:].rearrange("(do di) nh -> di do nh", di=P)
    wk_v = wk[:].rearrange("(do di) h -> di do h", di=P)
    wv_v = wv[:].rearrange("(do di) h -> di do h", di=P)
    wo_v = wo[:].rearrange("n h d -> h n d")

    with tile.TileContext(nc) as tc:
        with tc.tile_pool(name="glob", bufs=1) as glob:
            qt = glob.tile([P, G, T], F32R)
            kt = glob.tile([P, T], F32R)
            vsb = glob.tile([P, DB, H], F32R)
            tri_sb = glob.tile([P, P], F32)
            ones_col = glob.tile([P, 1], F32R)
            ones_row = glob.tile([1, P], F32R)
            ident = glob.tile([P, P], F32)
            pi_sb = glob.tile([P, P], F32R)
            nc.gpsimd.dma_start(pi_sb[:], pi[:])
            nc.gpsimd.dma_start(tri_sb[:], tri[:])
            nc.gpsimd.dma_start(ones_col[:], one_c[:])
            nc.gpsimd.dma_start(ones_row[:], one_r[:])
            make_identity(nc, ident[:])
            with tc.tile_pool(name="warm", bufs=1, space="PSUM") as wps:
                wtile = wps.tile([P, 16], F32, tag="warm")
                for _ in range(24):
                    nc.tensor.matmul(wtile[:], pi_sb[:], pi_sb[:, :16],
                                     start=True, stop=True)

            # ----- Phase 1+2 merged: per-chunk Q proj + RoPE, K proj + RoPE, V
            with tc.tile_pool(name="xp", bufs=30) as xp, \
                 tc.tile_pool(name="pw", bufs=1) as pw, \
                 tc.tile_pool(name="pt", bufs=3) as pt, \
                 tc.tile_pool(name="ps_proj", bufs=3, space="PSUM") as ps_proj, \
                 tc.tile_pool(name="ps_rot", bufs=2, space="PSUM") as ps_rot, \
                 tc.tile_pool(name="ps_tp", bufs=2, space="PSUM") as ps_tp:
                wq_sb = pw.tile([P, DB, G * H], F32R)
                wk_sb = pw.tile([P, DB, H], F32R)
                wv_sb = pw.tile([P, DB, H], F32R)
                cosq_sb = pw.tile([P, T], F32)
                sinq_sb = pw.tile([P, T], F32)
                for cc in range(NCH):
                    ccs = slice(cc * TCH, (cc + 1) * TCH)
                    nc.gpsimd.dma_start(cosq_sb[:, ccs], cq[:, ccs])
                    nc.gpsimd.dma_start(sinq_sb[:, ccs], sq[:, ccs])
                nc.gpsimd.dma_start(wk_sb[:], wk_v)
                nc.gpsimd.dma_start(wv_sb[:], wv_v)
                for c in range(NCH):
                    tsl = slice(c * TCH, (c + 1) * TCH)
                    # --- Q projection for this chunk
                    xts = []
                    for db in range(DB):
                        if c == 0:
                            nc.sync.dma_start(wq_sb[:, db], wq_v[:, db])
                        xt = xp.tile([P, TCH], F32R, tag="xt")
                        nc.sync.dma_start(xt[:], xq_v[:, db, tsl])
                        xts.append(xt)
                    for n in range(G):
                        ps = ps_proj.tile([P, TCH], F32, tag="proj")
                        for db in range(DB):
                            nc.tensor.matmul(
                                ps[:], wq_sb[:, db, n * H:(n + 1) * H],
                                xts[db][:],
                                start=(db == 0), stop=(db == DB - 1))
                        _rope_from_psum(nc, pt, ps_rot, ps, qt[:, n, tsl],
                                        cosq_sb[:, tsl], sinq_sb[:, tsl],
                                        pi_sb[:])
                    # --- K/V for this chunk
                    xts = []
                    for db in range(DB):
                        xt = xp.tile([P, TCH], F32R, tag="xt")
                        nc.sync.dma_start(xt[:], xkv_v[:, db, tsl])
                        xts.append(xt)
                    ps = ps_proj.tile([P, TCH], F32, tag="proj")
                    for db in range(DB):
                        nc.tensor.matmul(ps[:], wk_sb[:, db, :], xts[db][:],
                                         start=(db == 0), stop=(db == DB - 1))
                    _rope_from_psum(nc, pt, ps_rot, ps, kt[:, tsl],
                                    cosq_sb[:, tsl], sinq_sb[:, tsl], pi_sb[:])
                    ps2 = ps_proj.tile([P, TCH], F32, tag="proj")
                    for db in range(DB):
                        nc.tensor.matmul(ps2[:], wv_sb[:, db, :], xts[db][:],
                                         start=(db == 0), stop=(db == DB - 1))
                    vt_tmp = pt.tile([P, TCH], F32, tag="vt")
                    nc.scalar.copy(vt_tmp[:], ps2[:])
                    for kk in range(4):
                        pst = ps_tp.tile([P, P], F32, tag="tp")
                        nc.tensor.transpose(pst[:], vt_tmp[:, kk * P:(kk + 1) * P],
                                            ident[:])
                        nc.scalar.copy(vsb[:, 4 * c + kk, :], pst[:])

            # ---------- Phase 3: attention + o_proj ----------
            with tc.tile_pool(name="p3w", bufs=1) as p3w, \
                 tc.tile_pool(name="pb", bufs=8) as pbp, \
                 tc.tile_pool(name="otp", bufs=2) as otp, \
                 tc.tile_pool(name="smp", bufs=2) as smp, \
                 tc.tile_pool(name="obp", bufs=3) as obp, \
                 tc.tile_pool(name="ps_sc", bufs=3, space="PSUM") as ps_sc, \
                 tc.tile_pool(name="ps_ot", bufs=2, space="PSUM") as ps_ot, \
                 tc.tile_pool(name="ps_den", bufs=1, space="PSUM") as ps_den, \
                 tc.tile_pool(name="ps_o", bufs=2, space="PSUM") as ps_o:
                wo_sb = p3w.tile([P, G, D], F32R)
                nc.sync.dma_start(wo_sb[:], wo_v)
                for c in range(NCH):
                    tsl = slice(c * TCH, (c + 1) * TCH)
                    J = 4 * (c + 1)
                    otc = otp.tile([P, G, TCH], F32R, tag="otc")
                    for n in range(G):
                        ot_ps = ps_ot.tile([P, TCH], F32, tag="ot")
                        den_ps = ps_den.tile([1, TCH], F32, tag="den")
                        for j in range(J):
                            d = j - 4 * c
                            lo = max(d, 0) * P
                            s_ps = ps_sc.tile([P, TCH], F32, tag="sc")
                            nc.tensor.matmul(s_ps[:, lo:],
                                             kt[:, j * P:(j + 1) * P],
                                             qt[:, n, c * TCH + lo:(c + 1) * TCH],
                                             start=True, stop=True)
                            if d >= 0:
                                nc.vector.tensor_add(
                                    s_ps[:, d * P:(d + 1) * P],
                                    s_ps[:, d * P:(d + 1) * P], tri_sb[:])
                            pb = pbp.tile([P, TCH], F32R, tag="pb")
                            nc.scalar.activation(pb[:, lo:], s_ps[:, lo:], EXP)
                            nc.tensor.matmul(ot_ps[:, lo:], vsb[:, j, :],
                                             pb[:, lo:],
                                             start=(j == 0), stop=(j == J - 1))
                            nc.tensor.matmul(den_ps[:, lo:], ones_col[:],
                                             pb[:, lo:],
                                             start=(j == 0), stop=(j == J - 1))
                        den_sb = smp.tile([1, TCH], F32, tag="den_sb")
                        nc.vector.tensor_copy(den_sb[:], den_ps[:])
                        inv_f32 = smp.tile([1, TCH], F32, tag="inv_f32")
                        nc.vector.reciprocal_approx_fast(out=inv_f32[:],
                                                         in_=den_sb[:])
                        inv_sb = smp.tile([1, TCH], F32R, tag="inv_sb")
                        nc.vector.tensor_copy(inv_sb[:], inv_f32[:])
                        bc_ps = ps_o.tile([P, TCH], F32, tag="o")
                        nc.tensor.matmul(bc_ps[:], ones_row[:], inv_sb[:],
                                         start=True, stop=True)
                        invb = pbp.tile([P, TCH], F32, tag="invb")
                        nc.vector.tensor_copy(invb[:], bc_ps[:])
                        nc.vector.tensor_mul(otc[:, n, :], ot_ps[:], invb[:])
                    for kk in range(4):
                        row = c * TCH + kk * P
                        for dc in range(4):
                            ops = ps_o.tile([P, TCH], F32, tag="o")
                            for n in range(G):
                                nc.tensor.matmul(
                                    ops[:],
                                    otc[:, n, kk * P:(kk + 1) * P],
                                    wo_sb[:, n, dc * TCH:(dc + 1) * TCH],
                                    start=(n == 0), stop=(n == G - 1))
                            osb = obp.tile([P, TCH], F32, tag="osb")
                            nc.scalar.copy(osb[:], ops[:])
                            nc.sync.dma_start(
                                out[row:row + P, dc * TCH:(dc + 1) * TCH],
                                osb[:])

    nc.compile()
    _CACHE["nc"] = nc
    return nc


def _rope_tables(pos):
    ts = MIN_TS * (MAX_TS / MIN_TS) ** (2.0 * np.arange(HALF) / H)
    ang = pos.astype(np.float64)[None, :] / ts[:, None]
    c, s = np.cos(ang), np.sin(ang)
    cosF = np.ascontiguousarray(np.concatenate([c, c], 0).astype(np.float32))
    sinF = np.ascontiguousarray(np.concatenate([-s, s], 0).astype(np.float32))
    return cosF, sinF


def kernel(Xq, Xkv, q_positions, kv_positions, Wq, Wk, Wv, Wo, _trace=False):
    global LAST
    nc = _build()
    Xq = np.asarray(Xq, dtype=np.float32)
    Xkv = np.asarray(Xkv, dtype=np.float32)
    Wq = np.asarray(Wq, dtype=np.float32)
    Wk = np.asarray(Wk, dtype=np.float32)
    Wv = np.asarray(Wv, dtype=np.float32)
    Wo = np.asarray(Wo, dtype=np.float32)
    qp = np.asarray(q_positions)
    kp = np.asarray(kv_positions)
    assert np.array_equal(qp, kp), (
        "kernel assumes q_positions == kv_positions (RoPE tables shared)")

    idx = np.arange(P)
    tri_np = np.where(idx[:, None] <= idx[None, :], 0.0, NEG).astype(np.float32)
    pi_np = np.zeros((P, P), np.float32)
    pi_np[(idx + HALF) % P, idx] = 1.0

    in_maps = []
    for core in range(8):
        b, kv = divmod(core, 4)
        cq_, sq_ = _rope_tables(qp[b])
        in_maps.append({
            "xqT": np.ascontiguousarray(Xq[b].T),
            "xkvT": np.ascontiguousarray(Xkv[b].T),
            "wq": np.ascontiguousarray(
                Wq[:, kv * G:(kv + 1) * G, :].reshape(D, G * H)),
            "wk": np.ascontiguousarray(Wk[:, kv, :]),
            "wv": np.ascontiguousarray(Wv[:, kv, :]),
            "wo": np.ascontiguousarray(Wo[kv * G:(kv + 1) * G]),
            "cosq": cq_, "sinq": sq_,
            "tri": tri_np,
            "one_c": np.ones((P, 1), np.float32),
            "one_r": np.ones((1, P), np.float32),
            "pi": pi_np,
        })

    LAST = run_bass_kernel_spmd(nc, in_maps, list(range(8)), trace=_trace)
    parts = [r["O"] for r in LAST.results]
    O = np.stack([parts[0] + parts[1] + parts[2] + parts[3],
                  parts[4] + parts[5] + parts[6] + parts[7]])
    return np.ascontiguousarray(O.astype(np.float32))


# revision 24
# speedup vs baseline: 1.5891x; 1.0010x over previous
"""GQA causal-attention prefill (B=2, T=S=2048, D=2048, N=16, K=4, H=128)
on 8 Trainium2 NeuronCores.

Sharding: one (batch, kv-head) pair per core -> 2*4 = 8 cores, zero
cross-core communication on device; the o_proj partial sums (over each
batch's 4 kv-head groups) are reduced on the host at unshard time.

Per-core dataflow (all layouts chosen so no on-device transposes of the
big operands are ever needed; host pre-transposes Xq/Xkv once):
  QT_n[h,t] = Wq_n^T X^T        (matmul lhsT=Wq slice, rhs=XqT)  + RoPE
  KT[h,s]   = Wk^T Xkv^T                                          + RoPE
  VT[h,s]   = Wv^T Xkv^T  -> V[s,h] via PE transpose
  scoresT[s,t] = KT_blk^T QT    (lhsT=KT block, rhs=QT chunk)
  probsT = exp(scoresT + causal mask)            (ACT, unnormalized)
  OT_n[h,t] += V_blk^T probsT   ;  den[1,t] += ones^T probsT
  OT_n *= broadcast(1/den)      (ones-row matmul broadcast + DVE)
  O[t,d]  = sum_n OT_n^T Wo_n   (accumulated in PSUM over heads)

Matmuls run as float32r (full-rate fp32 on TRN2 for moving dim >= 256).
"""
import sys
import types

import numpy as np

try:  # make trace=True degrade gracefully when axon_hooks is absent
    import antenv.axon_hooks  # noqa: F401
except Exception:
    try:
        import antenv
        _m = types.ModuleType("antenv.axon_hooks")
        _h = [None]
        _m.set_axon_ntff_profile_hook = lambda h: _h.__setitem__(0, h)
        _m.get_axon_ntff_profile_hook = lambda: _h[0]
        sys.modules["antenv.axon_hooks"] = _m
        antenv.axon_hooks = _m
    except Exception:
        pass

import concourse.bass as bass
from concourse import bacc
import concourse.tile as tile
import concourse.mybir as mybir
from concourse.bass_utils import run_bass_kernel_spmd
from concourse.masks import make_identity

B, T, D = 2, 2048, 2048
N, K, H = 16, 4, 128
G = N // K
HALF = H // 2
MIN_TS, MAX_TS = 1.0, 10000.0

P = 128
TCH = 512
NCH = T // TCH          # 4 t-chunks of 512
DB = D // P             # 16 contraction blocks
F32 = mybir.dt.float32
F32R = mybir.dt.float32r
NEG = -1.0e30
EXP = mybir.ActivationFunctionType.Exp

_CACHE = {}
LAST = None             # BassKernelResults of the most recent run


def _rope_from_psum(nc, pool, pspool, ps, dst, cos_ap, sin_ap, pi_sb):
    """dst[128,TCH] = x*cosF + rot(x)*sinF; rot via PE permutation matmul."""
    nc.scalar.copy(dst, ps[:])
    rot_ps = pspool.tile([P, TCH], F32, tag="rotps")
    nc.tensor.matmul(rot_ps[:], pi_sb, dst, start=True, stop=True)
    prod = pool.tile([P, TCH], F32, tag="prod")
    nc.vector.tensor_mul(prod[:], rot_ps[:], sin_ap)
    nc.vector.tensor_mul(dst, dst, cos_ap)
    nc.vector.tensor_add(dst, dst, prod[:])


def _build():
    if "nc" in _CACHE:
        return _CACHE["nc"]
    nc = bacc.Bacc(None, target_bir_lowering=False, debug=False)
    xq = nc.declare_dram_parameter("xqT", [D, T], F32R, isOutput=False)
    xkv = nc.declare_dram_parameter("xkvT", [D, T], F32R, isOutput=False)
    wq = nc.declare_dram_parameter("wq", [D, G * H], F32R, isOutput=False)
    wk = nc.declare_dram_parameter("wk", [D, H], F32R, isOutput=False)
    wv = nc.declare_dram_parameter("wv", [D, H], F32R, isOutput=False)
    wo = nc.declare_dram_parameter("wo", [G, H, D], F32R, isOutput=False)
    cq = nc.declare_dram_parameter("cosq", [P, T], F32, isOutput=False)
    sq = nc.declare_dram_parameter("sinq", [P, T], F32, isOutput=False)
    tri = nc.declare_dram_parameter("tri", [P, P], F32, isOutput=False)
    one_c = nc.declare_dram_parameter("one_c", [P, 1], F32R, isOutput=False)
    one_r = nc.declare_dram_parameter("one_r", [1, P], F32R, isOutput=False)
    pi = nc.declare_dram_parameter("pi", [P, P], F32R, isOutput=False)
    out = nc.declare_dram_parameter("O", [T, D], F32, isOutput=True)

    xq_v = xq[:].rearrange("(do di) t -> di do t", di=P)
    xkv_v = xkv[:].rearrange("(do di) t -> di do t", di=P)
    wq_v = wq[:].rearrange("(do di) nh -> di do nh", di=P)
    wk_v = wk[:].rearrange("(do di) h -> di do h", di=P)
    wv_v = wv[:].rearrange("(do di) h -> di do h", di=P)
    wo_v = wo[:].rearrange("n h d -> h n d")

    with tile.TileContext(nc) as tc:
        with tc.tile_pool(name="glob", bufs=1) as glob:
            qt = glob.tile([P, G, T], F32R)
            kt = glob.tile([P, T], F32R)
            vsb = glob.tile([P, DB, H], F32R)
            tri_sb = glob.tile([P, P], F32)
            ones_col = glob.tile([P, 1], F32R)
            ones_row = glob.tile([1, P], F32R)
            ident = glob.tile([P, P], F32)
            pi_sb = glob.tile([P, P], F32R)
            nc.gpsimd.dma_start(pi_sb[:], pi[:])
            nc.gpsimd.dma_start(tri_sb[:], tri[:])
            nc.gpsimd.dma_start(ones_col[:], one_c[:])
            nc.gpsimd.dma_start(ones_row[:], one_r[:])
            make_identity(nc, ident[:])
            with tc.tile_pool(name="warm", bufs=1, space="PSUM") as wps:
                wtile = wps.tile([P, 16], F32, tag="warm")
                for _ in range(24):
                    nc.tensor.matmul(wtile[:], pi_sb[:], pi_sb[:, :16],
                                     start=True, stop=True)

            # ----- Phase 1+2 merged: per-chunk Q proj + RoPE, K proj + RoPE, V
            with tc.tile_pool(name="xp", bufs=30) as xp, \
                 tc.tile_pool(name="pw", bufs=1) as pw, \
                 tc.tile_pool(name="pt", bufs=3) as pt, \
                 tc.tile_pool(name="ps_proj", bufs=3, space="PSUM") as ps_proj, \
                 tc.tile_pool(name="ps_rot", bufs=2, space="PSUM") as ps_rot, \
                 tc.tile_pool(name="ps_tp", bufs=2, space="PSUM") as ps_tp:
                wq_sb = pw.tile([P, DB, G * H], F32R)
                wk_sb = pw.tile([P, DB, H], F32R)
                wv_sb = pw.tile([P, DB, H], F32R)
                cosq_sb = pw.tile([P, T], F32)
                sinq_sb = pw.tile([P, T], F32)
                for cc in range(NCH):
                    ccs = slice(cc * TCH, (cc + 1) * TCH)
                    nc.gpsimd.dma_start(cosq_sb[:, ccs], cq[:, ccs])
                    nc.gpsimd.dma_start(sinq_sb[:, ccs], sq[:, ccs])
                nc.gpsimd.dma_start(wk_sb[:], wk_v)
                nc.gpsimd.dma_start(wv_sb[:], wv_v)
                for c in range(NCH):
                    tsl = slice(c * TCH, (c + 1) * TCH)
                    # --- Q projection for this chunk
                    xts = []
                    for db in range(DB):
                        if c == 0:
                            nc.sync.dma_start(wq_sb[:, db], wq_v[:, db])
                        xt = xp.tile([P, TCH], F32R, tag="xt")
                        nc.sync.dma_start(xt[:], xq_v[:, db, tsl])
                        xts.append(xt)
                    for n in range(G):
                        ps = ps_proj.tile([P, TCH], F32, tag="proj")
                        for db in range(DB):
                            nc.tensor.matmul(
                                ps[:], wq_sb[:, db, n * H:(n + 1) * H],
                                xts[db][:],
                                start=(db == 0), stop=(db == DB - 1))
                        _rope_from_psum(nc, pt, ps_rot, ps, qt[:, n, tsl],
                                        cosq_sb[:, tsl], sinq_sb[:, tsl],
                                        pi_sb[:])
                    # --- K/V for this chunk
                    xts = []
                    for db in range(DB):
                        xt = xp.tile([P, TCH], F32R, tag="xt")
                        nc.sync.dma_start(xt[:], xkv_v[:, db, tsl])
                        xts.append(xt)
                    ps = ps_proj.tile([P, TCH], F32, tag="proj")
                    for db in range(DB):
                        nc.tensor.matmul(ps[:], wk_sb[:, db, :], xts[db][:],
                                         start=(db == 0), stop=(db == DB - 1))
                    _rope_from_psum(nc, pt, ps_rot, ps, kt[:, tsl],
                                    cosq_sb[:, tsl], sinq_sb[:, tsl], pi_sb[:])
                    ps2 = ps_proj.tile([P, TCH], F32, tag="proj")
                    for db in range(DB):
                        nc.tensor.matmul(ps2[:], wv_sb[:, db, :], xts[db][:],
                                         start=(db == 0), stop=(db == DB - 1))
                    vt_tmp = pt.tile([P, TCH], F32, tag="vt")
                    nc.scalar.copy(vt_tmp[:], ps2[:])
                    for kk in range(4):
                        pst = ps_tp.tile([P, P], F32, tag="tp")
                        nc.tensor.transpose(pst[:], vt_tmp[:, kk * P:(kk + 1) * P],
                                            ident[:])
                        nc.scalar.copy(vsb[:, 4 * c + kk, :], pst[:])

            # ---------- Phase 3: attention + o_proj ----------
            with tc.tile_pool(name="p3w", bufs=1) as p3w, \
                 tc.tile_pool(name="pb", bufs=10) as pbp, \
                 tc.tile_pool(name="otp", bufs=2) as otp, \
                 tc.tile_pool(name="smp", bufs=2) as smp, \
                 tc.tile_pool(name="obp", bufs=3) as obp, \
                 tc.tile_pool(name="ps_sc", bufs=3, space="PSUM") as ps_sc, \
                 tc.tile_pool(name="ps_ot", bufs=2, space="PSUM") as ps_ot, \
                 tc.tile_pool(name="ps_den", bufs=1, space="PSUM") as ps_den, \
                 tc.tile_pool(name="ps_o", bufs=2, space="PSUM") as ps_o:
                wo_sb = p3w.tile([P, G, D], F32R)
                nc.sync.dma_start(wo_sb[:], wo_v)
                for c in range(NCH):
                    tsl = slice(c * TCH, (c + 1) * TCH)
                    J = 4 * (c + 1)
                    otc = otp.tile([P, G, TCH], F32R, tag="otc")
                    for n in range(G):
                        ot_ps = ps_ot.tile([P, TCH], F32, tag="ot")
                        den_ps = ps_den.tile([1, TCH], F32, tag="den")
                        for j in range(J):
                            d = j - 4 * c
                            lo = max(d, 0) * P
                            s_ps = ps_sc.tile([P, TCH], F32, tag="sc")
                            nc.tensor.matmul(s_ps[:, lo:],
                                             kt[:, j * P:(j + 1) * P],
                                             qt[:, n, c * TCH + lo:(c + 1) * TCH],
                                             start=True, stop=True)
                            if d >= 0:
                                nc.vector.tensor_add(
                                    s_ps[:, d * P:(d + 1) * P],
                                    s_ps[:, d * P:(d + 1) * P], tri_sb[:])
                            pb = pbp.tile([P, TCH], F32R, tag="pb")
                            nc.scalar.activation(pb[:, lo:], s_ps[:, lo:], EXP)
                            nc.tensor.matmul(ot_ps[:, lo:], vsb[:, j, :],
                                             pb[:, lo:],
                                             start=(j == 0), stop=(j == J - 1))
                            nc.tensor.matmul(den_ps[:, lo:], ones_col[:],
                                             pb[:, lo:],
                                             start=(j == 0), stop=(j == J - 1))
                        den_sb = smp.tile([1, TCH], F32, tag="den_sb")
                        nc.vector.tensor_copy(den_sb[:], den_ps[:])
                        inv_f32 = smp.tile([1, TCH], F32, tag="inv_f32")
                        nc.vector.reciprocal_approx_fast(out=inv_f32[:],
                                                         in_=den_sb[:])
                        inv_sb = smp.tile([1, TCH], F32R, tag="inv_sb")
                        nc.vector.tensor_copy(inv_sb[:], inv_f32[:])
                        bc_ps = ps_o.tile([P, TCH], F32, tag="o")
                        nc.tensor.matmul(bc_ps[:], ones_row[:], inv_sb[:],
                                         start=True, stop=True)
                        invb = pbp.tile([P, TCH], F32, tag="invb")
                        nc.vector.tensor_copy(invb[:], bc_ps[:])
                        nc.vector.tensor_mul(otc[:, n, :], ot_ps[:], invb[:])
                    for kk in range(4):
                        row = c * TCH + kk * P
                        for dc in range(4):
                            ops = ps_o.tile([P, TCH], F32, tag="o")
                            for n in range(G):
                                nc.tensor.matmul(
                                    ops[:],
                                    otc[:, n, kk * P:(kk + 1) * P],
                                    wo_sb[:, n, dc * TCH:(dc + 1) * TCH],
                                    start=(n == 0), stop=(n == G - 1))
                            osb = obp.tile([P, TCH], F32, tag="osb")
                            nc.scalar.copy(osb[:], ops[:])
                            nc.sync.dma_start(
                                out[row:row + P, dc * TCH:(dc + 1) * TCH],
                                osb[:])

    nc.compile()
    _CACHE["nc"] = nc
    return nc


def _rope_tables(pos):
    ts = MIN_TS * (MAX_TS / MIN_TS) ** (2.0 * np.arange(HALF) / H)
    ang = pos.astype(np.float64)[None, :] / ts[:, None]
    c, s = np.cos(ang), np.sin(ang)
    cosF = np.ascontiguousarray(np.concatenate([c, c], 0).astype(np.float32))
    sinF = np.ascontiguousarray(np.concatenate([-s, s], 0).astype(np.float32))
    return cosF, sinF


def kernel(Xq, Xkv, q_positions, kv_positions, Wq, Wk, Wv, Wo, _trace=False):
    global LAST
    nc = _build()
    Xq = np.asarray(Xq, dtype=np.float32)
    Xkv = np.asarray(Xkv, dtype=np.float32)
    Wq = np.asarray(Wq, dtype=np.float32)
    Wk = np.asarray(Wk, dtype=np.float32)
    Wv = np.asarray(Wv, dtype=np.float32)
    Wo = np.asarray(Wo, dtype=np.float32)
    qp = np.asarray(q_positions)
    kp = np.asarray(kv_positions)
    assert np.array_equal(qp, kp), (
        "kernel assumes q_positions == kv_positions (RoPE tables shared)")

    idx = np.arange(P)
    tri_np = np.where(idx[:, None] <= idx[None, :], 0.0, NEG).astype(np.float32)
    pi_np = np.zeros((P, P), np.float32)
    pi_np[(idx + HALF) % P, idx] = 1.0

    in_maps = []
    for core in range(8):
        b, kv = divmod(core, 4)
        cq_, sq_ = _rope_tables(qp[b])
        in_maps.append({
            "xqT": np.ascontiguousarray(Xq[b].T),
            "xkvT": np.ascontiguousarray(Xkv[b].T),
            "wq": np.ascontiguousarray(
                Wq[:, kv * G:(kv + 1) * G, :].reshape(D, G * H)),
            "wk": np.ascontiguousarray(Wk[:, kv, :]),
            "wv": np.ascontiguousarray(Wv[:, kv, :]),
            "wo": np.ascontiguousarray(Wo[kv * G:(kv + 1) * G]),
            "cosq": cq_, "sinq": sq_,
            "tri": tri_np,
            "one_c": np.ones((P, 1), np.float32),
            "one_r": np.ones((1, P), np.float32),
            "pi": pi_np,
        })

    LAST = run_bass_kernel_spmd(nc, in_maps, list(range(8)), trace=_trace)
    parts = [r["O"] for r in LAST.results]
    O = np.stack([parts[0] + parts[1] + parts[2] + parts[3],
                  parts[4] + parts[5] + parts[6] + parts[7]])
    return np.ascontiguousarray(O.astype(np.float32))
